# revision 14
# baseline (speedup 1.0000x reference)
"""EquivariantGraphConvolution (EGNN layer) on 8 Trainium2 NeuronCores.

Strategy (v2 — streamed, gather-free)
-------------------------------------
Nodes are range-partitioned across the 8 cores (6250 each); every edge is owned
by the core that owns its *start* node, so per-start segment sums are
core-local and no collective is needed.

Per core, edges are sorted by 128-node start block and padded per block to a
uniform CPB chunks of 128 edges.  The host pre-gathers both endpoints' node
features per edge and stages them as sequentially-streamed feature-major bf16
tensors (plus edge features / dist / coords-diff / lane ids), so the device
does ZERO indirect DMA — the edge MLP is pure dense matmul work:

  x1[128,512] = W1s_bd.T@nfs + W1e_bd.T@nfe + Wef.T@efcdn   (2 edge groups
  feature-stacked on partitions), SiLU chains for message/coords nets, a merged
  transpose+gate+coordw matmul per 128-edge chunk (K=128: msg|coord stacked),
  and a one-hot matmul segment-sum per chunk with the one-hot stationary
  (out is node-major [128,67] = 64 msg-agg + 3 coord-agg).

Node updates (velocity/node MLPs, coordinate update) run on-chip afterwards.
"""
import sys
sys.path.insert(0, "/opt/trn_rl_repo")
import contextlib
import numpy as np
import ml_dtypes

import concourse.bass as bass
import concourse.bacc as bacc
import concourse.mybir as mybir
import concourse.tile as tile
from concourse.bass_utils import run_bass_kernel_spmd

f32 = mybir.dt.float32
bf16 = mybir.dt.bfloat16
i32 = mybir.dt.int32
AF = mybir.ActivationFunctionType
OP = mybir.AluOpType
BF = ml_dtypes.bfloat16

# ---- problem constants (hardcoded per contract) ----
N = 50000
E = 1_000_000
H = 64
EF = 16
NCORES = 8
NPC = N // NCORES          # 6250 nodes per core
NBLK = 49                  # 128-node blocks per core (49*128 = 6272 >= 6250)
NPAD = NBLK * 128          # 6272

_cache = {}


def _f(x):
    return np.ascontiguousarray(x, np.float32)


def _bd(W):
    """[64,64] -> [128,128] block diagonal."""
    out = np.zeros((128, 128), np.float32)
    out[0:64, 0:64] = W
    out[64:128, 64:128] = W
    return out


def _prep_weights(inp):
    """Small weight/constant tensors, identical on all cores."""
    W_e1 = _f(inp["W_e1"])           # [145, 64]
    w = {}
    w["w1s"] = _bd(W_e1[0:64]).astype(BF)
    w["w1e"] = _bd(W_e1[64:128]).astype(BF)
    wef = np.zeros((34, 128), np.float32)
    wef[0:16, 0:64] = W_e1[129:145]
    wef[16:17, 0:64] = W_e1[128:129]
    wef[17:33, 64:128] = W_e1[129:145]
    wef[33:34, 64:128] = W_e1[128:129]
    w["wefcdn"] = wef.astype(BF)
    w["wde2"] = _bd(_f(inp["W_e2"])).astype(BF)
    w["wdc1"] = _bd(_f(inp["W_c1"])).astype(BF)
    W_i = _f(inp["W_i"]); W_c2 = _f(inp["W_c2"])
    ra = np.zeros((128, 65), np.float32)
    ra[0:64, 0:64] = np.eye(64); ra[0:64, 64:65] = W_i
    ra[64:128, 0:64] = np.eye(64); ra[64:128, 64:65] = W_i
    w["raw2"] = ra.astype(BF)
    wc2r = np.zeros((128, 1), np.float32)
    wc2r[0:64] = W_c2; wc2r[64:128] = W_c2
    w["wc2r"] = wc2r.astype(BF)
    w["wn1"] = _f(inp["W_n1"])       # [128, 64]
    w["wn2"] = _f(inp["W_n2"])       # [64, 64]
    w["wv1"] = _f(inp["W_v1"])       # [64, 64]
    w["wv2"] = _f(inp["W_v2"])       # [64, 1]
    w["eye64"] = np.eye(64, dtype=np.float32)
    w["eye128"] = np.eye(128, dtype=np.float32)
    b_e1 = _f(inp["b_e1"]); b_e2 = _f(inp["b_e2"]); b_c1 = _f(inp["b_c1"])
    w["be1s"] = np.concatenate([b_e1, b_e1]).reshape(128, 1)
    w["be2s"] = np.concatenate([b_e2, b_e2]).reshape(128, 1)
    w["bc1s"] = np.concatenate([b_c1, b_c1]).reshape(128, 1)
    w["bih"] = np.full((128, 1), 0.5 * float(np.asarray(inp["b_i"]).ravel()[0]), np.float32)
    w["bn1c"] = _f(inp["b_n1"]).reshape(64, 1)
    w["bn2c"] = _f(inp["b_n2"]).reshape(64, 1)
    w["bv1c"] = _f(inp["b_v1"]).reshape(64, 1)
    w["bv2c"] = np.full((1, 1), float(np.asarray(inp["b_v2"]).ravel()[0]), np.float32)
    return w


WSHAPES = {"w1s": [128, 128], "w1e": [128, 128], "wefcdn": [34, 128],
           "wde2": [128, 128], "wdc1": [128, 128], "raw2": [128, 65],
           "wc2r": [128, 1],
           "wn1": [128, 64], "wn2": [64, 64], "wv1": [64, 64],
           "wv2": [64, 1], "eye64": [64, 64], "eye128": [128, 128],
           "be1s": [128, 1], "be2s": [128, 1], "bc1s": [128, 1],
           "bih": [128, 1], "bn1c": [64, 1], "bn2c": [64, 1],
           "bv1c": [64, 1], "bv2c": [1, 1]}
BF_W = ("w1s", "w1e", "wefcdn", "wde2", "wdc1", "raw2", "wc2r")


def _prep_core(c, start, end, ef, nfi, cd_all, cdn_all, invcnt_all, CPB, S):
    """Per-core staged edge streams (sorted by start block, block-padded)."""
    NCH = S * 8
    NSLOT = NCH * 128
    lo, hi = c * NPC, (c + 1) * NPC
    sel = (start >= lo) & (start < hi)
    eo = np.nonzero(sel)[0]
    s_loc = (start[eo] - lo).astype(np.int64)
    blk = s_loc >> 7
    order = np.argsort(blk, kind="stable")
    eo = eo[order]; s_loc = s_loc[order]; blk = blk[order]
    counts = np.bincount(blk, minlength=NBLK)
    if counts.max() > CPB * 128:
        raise RuntimeError(f"block overflow: {counts.max()} > {CPB * 128}")
    starts = np.zeros(NBLK, np.int64)
    starts[1:] = np.cumsum(counts)[:-1]
    within = np.arange(len(eo)) - starts[blk]
    slots = blk * (CPB * 128) + within

    nf64 = nfi[:, 6:70]
    nfs_sl = np.zeros((NSLOT, 64), np.float32)
    nfe_sl = np.zeros((NSLOT, 64), np.float32)
    ef_sl = np.zeros((NSLOT, EF), np.float32)
    cdn_sl = np.zeros(NSLOT, np.float32)
    cd_sl = np.zeros((NSLOT, 3), np.float32)
    lid_sl = np.full(NSLOT, -1.0, np.float32)
    nfs_sl[slots] = nf64[start[eo]]
    nfe_sl[slots] = nf64[end[eo]]
    ef_sl[slots] = ef[eo]
    cdn_sl[slots] = cdn_all[eo]
    cd_sl[slots] = cd_all[eo]
    lid_sl[slots] = (s_loc & 127).astype(np.float32)

    d = {}
    # feature-major, 2 edge groups of 512 stacked on partitions;
    # nfse packs [nfs | nfe] along the free dim -> one DMA per supertile
    nfse = np.empty((S, 128, 1024), BF)
    v = nfs_sl.reshape(S, 2, 512, 64).transpose(0, 1, 3, 2)
    nfse[:, :, 0:512] = v.reshape(S, 128, 512)
    v = nfe_sl.reshape(S, 2, 512, 64).transpose(0, 1, 3, 2)
    nfse[:, :, 512:1024] = v.reshape(S, 128, 512)
    d["nfse"] = nfse
    eft = ef_sl.reshape(S, 2, 512, EF).transpose(0, 1, 3, 2)   # [S,2,16,512]
    cdnr = cdn_sl.reshape(S, 2, 512)
    d["efcdn"] = np.concatenate(
        [eft[:, 0], cdnr[:, 0][:, None, :], eft[:, 1], cdnr[:, 1][:, None, :]],
        axis=1).astype(BF)                                      # [S,34,512]
    cdlid = np.empty((S, 128, 8, 4), BF)
    cdlid[:, :, :, 0:3] = cd_sl.reshape(S, 8, 128, 3).transpose(0, 2, 1, 3)
    cdlid[:, :, :, 3] = lid_sl.reshape(S, 8, 128).transpose(0, 2, 1)
    d["cdlid"] = cdlid

    nm = np.zeros((NPAD, 70), np.float32)
    nm[0:NPC] = nfi[lo:hi]
    d["node_nm"] = nm.reshape(NBLK, 128, 70).transpose(1, 0, 2).reshape(128, NBLK * 70).copy()
    ic = np.ones(NPAD, np.float32)
    ic[0:NPC] = invcnt_all[lo:hi]
    d["invcnt"] = ic.reshape(NBLK, 128).T.copy()                # [128, NBLK]
    nl = np.zeros((64, NPAD), np.float32)
    nl[:, 0:NPC] = nfi[lo:hi, 6:70].T
    d["nfT_local"] = nl
    return d


def _build_program(CPB, S):
    NCH = S * 8
    nc = bacc.Bacc("TRN2", target_bir_lowering=False, debug=False,
                   enable_asserts=False, num_devices=NCORES)

    def din(name, shape, dt=f32):
        return nc.dram_tensor(name, list(shape), dt, kind="ExternalInput").ap()

    nfse_d = din("nfse", [S, 128, 1024], bf16)
    efcdn_d = din("efcdn", [S, 34, 512], bf16)
    cdlid_d = din("cdlid", [S, 128, 8, 4], bf16)
    invcnt_d = din("invcnt", [128, NBLK])
    node_nm_d = din("node_nm", [128, NBLK * 70])
    nfT_loc_d = din("nfT_local", [64, NPAD])
    wd = {n: din(n, WSHAPES[n], bf16 if n in BF_W else f32) for n in WSHAPES}
    out_d = nc.dram_tensor("out", [NPAD, 70], f32, kind="ExternalOutput").ap()

    with tile.TileContext(nc) as tc, contextlib.ExitStack() as ctx:
        wpool = ctx.enter_context(tc.tile_pool(name="w", bufs=1))
        wt = {}
        for n in WSHAPES:
            t = wpool.tile(WSHAPES[n], bf16 if n in BF_W else f32, name=f"wt_{n}")
            nc.sync.dma_start(t[:], wd[n][:])
            wt[n] = t
        iota32 = wpool.tile([128, 128], i32, name="iota32")
        nc.gpsimd.iota(iota32[:], pattern=[[1, 128]], base=0, channel_multiplier=0)
        iota = wpool.tile([128, 128], bf16, name="iota")
        nc.vector.tensor_copy(iota[:], iota32[:])
        node_nm = wpool.tile([128, NBLK * 70], f32, name="node_nm")
        nc.sync.dma_start(node_nm[:], node_nm_d[:])
        invcnt = wpool.tile([128, NBLK], f32, name="invcnt")
        nc.sync.dma_start(invcnt[:], invcnt_d[:])
        nfT_loc = wpool.tile([64, NPAD], f32, name="nfT_loc")
        nc.sync.dma_start(nfT_loc[:], nfT_loc_d[:])
        vscale = wpool.tile([128, NBLK], f32, name="vscale")
        aggsb = wpool.tile([128, NBLK * 67], f32, name="aggsb")  # node-major [n, blk*67]

        # ---------- Phase B: velocity MLP -> vscale [128, NBLK] ----------
        with tc.tile_pool(name="pb", bufs=2) as pb, \
             tc.tile_pool(name="pbp", bufs=2, space="PSUM") as pbp:
            tiles = [(j * 512, 512) for j in range(NPAD // 512)]
            if NPAD % 512:
                tiles.append((NPAD // 512 * 512, NPAD % 512))
            for (o, L) in tiles:
                vps = pbp.tile([64, L], f32, name=f"vps{o}", tag="vps")
                nc.tensor.matmul(vps[:], wt["wv1"][:], nfT_loc[:, o:o + L])
                vh = pb.tile([64, L], f32, name=f"vh{o}", tag="vh")
                nc.scalar.activation(vh[:], vps[:], AF.Silu, bias=wt["bv1c"][:])
                sps = pbp.tile([1, L], f32, name=f"sps{o}", tag="sps")
                nc.tensor.matmul(sps[:], wt["wv2"][:], vh[:])
                vsc = pb.tile([1, L], f32, name=f"vsc{o}", tag="vsc")
                nc.scalar.activation(vsc[:], sps[:], AF.Identity, bias=wt["bv2c"][:])
                for k in range(L // 128):
                    tp = pbp.tile([128, 1], f32, name=f"tp{o}_{k}", tag="tp")
                    nc.tensor.transpose(tp[:], vsc[:, k * 128:(k + 1) * 128],
                                        wt["eye64"][0:1, 0:1])
                    nc.vector.tensor_copy(vscale[:, o // 128 + k:o // 128 + k + 1], tp[:])

        # ---------- Edge sweep ----------
        with tc.tile_pool(name="pin", bufs=6) as pin, \
             tc.tile_pool(name="pmid", bufs=4) as pmid, \
             tc.tile_pool(name="px", bufs=2, space="PSUM") as px, \
             tc.tile_pool(name="pm", bufs=2, space="PSUM") as pm, \
             tc.tile_pool(name="pst", bufs=2, space="PSUM") as pst, \
             tc.tile_pool(name="pagg", bufs=2, space="PSUM") as pagg:
            aggN = None
            live = {}
            # 3-stage software pipeline: A(s) load+edge-MLP-front,
            # B(s-1) coords-net+transpose+gate, C(s-2) one-hot scatter.
            # Keeps the PE queue stocked with ready work so the HAM clock
            # gate stays at 8/8.
            for s in range(S + 2):
                if s < S:
                    lv = live[s] = {}
                    nfse = pin.tile([128, 1024], bf16, name=f"nfse{s}", tag="nfse")
                    nc.sync.dma_start(nfse[:], nfse_d[s])
                    eft = pin.tile([34, 512], bf16, name=f"eft{s}", tag="eft")
                    nc.sync.dma_start(eft[:], efcdn_d[s])
                    cdlid = pin.tile([128, 8, 4], bf16, name=f"cdlid{s}", tag="cdlid")
                    nc.sync.dma_start(cdlid[:], cdlid_d[s])
                    lv["cdlid"] = cdlid

                    oht = pmid.tile([128, 8, 128], bf16, name=f"oht{s}", tag="oht")
                    nc.vector.tensor_tensor(
                        oht[:], iota[:].unsqueeze(1).broadcast_to([128, 8, 128]),
                        cdlid[:, :, 3:4].broadcast_to([128, 8, 128]), OP.is_equal)
                    lv["oht"] = oht

                    x1 = px.tile([128, 512], f32, name=f"x1{s}", tag="x1")
                    nc.tensor.matmul(x1[:], wt["w1s"][:], nfse[:, 0:512],
                                     start=True, stop=False)
                    nc.tensor.matmul(x1[:], wt["w1e"][:], nfse[:, 512:1024],
                                     start=False, stop=False)
                    nc.tensor.matmul(x1[:], wt["wefcdn"][:], eft[:],
                                     start=False, stop=True)
                    h1 = pmid.tile([128, 512], bf16, name=f"h1{s}", tag="h1")
                    nc.scalar.activation(h1[:], x1[:], AF.Silu, bias=wt["be1s"][:])
                    mp = pm.tile([128, 512], f32, name=f"mp{s}", tag="mm2")
                    nc.tensor.matmul(mp[:], wt["wde2"][:], h1[:])
                    msgT = pmid.tile([128, 512], bf16, name=f"msgT{s}", tag="msgT")
                    nc.scalar.activation(msgT[:], mp[:], AF.Silu, bias=wt["be2s"][:])
                    lv["msgT"] = msgT

                if 1 <= s:
                    t = s - 1
                    if t < S:
                        lv = live[t]
                        msgT = lv["msgT"]; cdlid = lv["cdlid"]
                        cp = pm.tile([128, 512], f32, name=f"cp{t}", tag="mm2")
                        nc.tensor.matmul(cp[:], wt["wdc1"][:], msgT[:])
                        chT = pmid.tile([128, 512], bf16, name=f"chT{t}", tag="chT")
                        nc.scalar.activation(chT[:], cp[:], AF.Silu, bias=wt["bc1s"][:])
                        sts = []
                        for g in range(2):
                            rows = slice(g * 64, g * 64 + 64)
                            st = pst.tile([128, 4, 66], f32, name=f"st{t}_{g}", tag="st")
                            sts.append(st)
                            for c4 in range(4):
                                cc = slice(c4 * 128, (c4 + 1) * 128)
                                nc.tensor.matmul(st[:, c4, 0:65], msgT[rows, cc],
                                                 wt["raw2"][rows, :],
                                                 start=True, stop=True)
                        for g in range(2):
                            rows = slice(g * 64, g * 64 + 64)
                            st = sts[g]
                            for c4 in range(4):
                                cc = slice(c4 * 128, (c4 + 1) * 128)
                                nc.tensor.matmul(st[:, c4, 65:66], chT[rows, cc],
                                                 wt["wc2r"][rows, :],
                                                 start=True, stop=True)
                        rgc = pmid.tile([128, 8, 67], bf16, name=f"rgc{t}", tag="rgc")
                        lv["rgc"] = rgc
                        for g in range(2):
                            st = sts[g]
                            tnh = pmid.tile([128, 4], f32, name=f"tnh{t}_{g}", tag="tnh")
                            nc.scalar.activation(tnh[:], st[:, :, 64:65].squeeze(2),
                                                 AF.Tanh, bias=wt["bih"][:], scale=0.5)
                            gate = pmid.tile([128, 4], f32, name=f"gt{t}_{g}", tag="gate")
                            nc.vector.tensor_scalar(out=gate[:], in0=tnh[:], scalar1=1.0,
                                                    scalar2=0.5, op0=OP.add, op1=OP.mult)
                            gsl = slice(g * 4, g * 4 + 4)
                            nc.vector.tensor_tensor(
                                rgc[:, gsl, 0:64], st[:, :, 0:64],
                                gate[:].unsqueeze(2).broadcast_to([128, 4, 64]), OP.mult)
                            nc.vector.tensor_tensor(
                                rgc[:, gsl, 64:67], cdlid[:, gsl, 0:3],
                                st[:, :, 65:66].broadcast_to([128, 4, 3]), OP.mult)

                if s >= 2:
                    u = s - 2
                    lv = live[u]
                    oht = lv["oht"]; rgc = lv["rgc"]
                    for k in range(8):
                        gc = u * 8 + k
                        vb = gc // CPB
                        if vb >= NBLK:
                            continue
                        pos = gc % CPB
                        if pos == 0:
                            aggN = pagg.tile([128, 128], f32, name=f"agg{vb}", tag="agg")
                        nc.tensor.matmul(aggN[:, 0:67], oht[:, k, :], rgc[:, k, :],
                                         start=(pos == 0), stop=(pos == CPB - 1),
                                         skip_group_check=True)
                        if pos == CPB - 1:
                            nc.vector.tensor_copy(aggsb[:, vb * 67:(vb + 1) * 67],
                                                  aggN[:, 0:67])
                    del live[u]

        # ---------- Phase C: node update + output ----------
        with tc.tile_pool(name="pc", bufs=3) as pc, \
             tc.tile_pool(name="pcp", bufs=2, space="PSUM") as pcp:
            b0 = 0
            while b0 < NBLK:
                BB = min(4, NBLK - b0)
                L = BB * 128
                xnT = pc.tile([128, BB, 128], f32, name=f"xnT{b0}", tag="xnT")
                nc.vector.tensor_copy(
                    xnT[0:64, :, :],
                    nfT_loc[:, b0 * 128:b0 * 128 + L].rearrange(
                        "p (b n) -> p b n", b=BB))
                atp = pcp.tile([64, BB, 128], f32, name=f"atp{b0}", tag="atp")
                for j in range(BB):
                    nc.tensor.transpose(
                        atp[:, j, :],
                        aggsb[:, (b0 + j) * 67:(b0 + j) * 67 + 64],
                        wt["eye128"][:])
                nc.vector.tensor_copy(xnT[64:128, :, :], atp[:])
                n1 = pcp.tile([64, BB, 128], f32, name=f"n1{b0}", tag="n1")
                nc.tensor.matmul(n1[:].rearrange("p b n -> p (b n)"), wt["wn1"][:],
                                 xnT[:].rearrange("p b n -> p (b n)"))
                hn = pc.tile([64, BB, 128], f32, name=f"hn{b0}", tag="hn")
                nc.scalar.activation(hn[:].rearrange("p b n -> p (b n)"),
                                     n1[:].rearrange("p b n -> p (b n)"),
                                     AF.Silu, bias=wt["bn1c"][:])
                n2 = pcp.tile([64, BB, 128], f32, name=f"n2{b0}", tag="n2")
                nc.tensor.matmul(n2[:].rearrange("p b n -> p (b n)"), wt["wn2"][:],
                                 hn[:].rearrange("p b n -> p (b n)"))
                hn2 = pc.tile([64, BB, 128], f32, name=f"hn2{b0}", tag="hn2")
                nc.scalar.activation(hn2[:].rearrange("p b n -> p (b n)"),
                                     n2[:].rearrange("p b n -> p (b n)"),
                                     AF.Identity, bias=wt["bn2c"][:])
                ndel = pcp.tile([128, BB, 64], f32, name=f"ndel{b0}", tag="ndel")
                for j in range(BB):
                    nc.tensor.transpose(ndel[:, j, :], hn2[:, j, :], wt["eye64"][:])
                nmb = node_nm[:, b0 * 70:(b0 + BB) * 70].rearrange(
                    "p (b f) -> p b f", b=BB)
                t1 = pc.tile([128, BB, 3], f32, name=f"t1{b0}", tag="t1")
                nc.vector.tensor_tensor(
                    t1[:],
                    aggsb[:, b0 * 67:(b0 + BB) * 67].rearrange(
                        "p (b f) -> p b f", b=BB)[:, :, 64:67],
                    invcnt[:, b0:b0 + BB].unsqueeze(2).broadcast_to([128, BB, 3]),
                    OP.mult)
                t2 = pc.tile([128, BB, 3], f32, name=f"t2{b0}", tag="t2")
                nc.vector.tensor_tensor(
                    t2[:], nmb[:, :, 3:6],
                    vscale[:, b0:b0 + BB].unsqueeze(2).broadcast_to([128, BB, 3]),
                    OP.mult)
                t3 = pc.tile([128, BB, 3], f32, name=f"t3{b0}", tag="t3")
                nc.vector.tensor_tensor(t3[:], t1[:], t2[:], OP.add)
                ot = pc.tile([128, BB, 70], f32, name=f"ot{b0}", tag="ot")
                nc.vector.tensor_tensor(ot[:, :, 0:3], t3[:], nmb[:, :, 0:3], OP.add)
                nc.vector.tensor_copy(ot[:, :, 3:6], nmb[:, :, 3:6])
                nc.vector.tensor_tensor(ot[:, :, 6:70], nmb[:, :, 6:70], ndel[:],
                                        OP.add)
                nc.sync.dma_start(
                    out_d[b0 * 128:(b0 + BB) * 128, :].rearrange(
                        "(b p) f -> p b f", p=128),
                    ot[:])
                b0 += BB

    nc.compile()
    return nc


def kernel(**inputs):
    ei = np.asarray(inputs["edge_indices"])
    start = ei[0].astype(np.int64)
    end = ei[1].astype(np.int64)
    ef = _f(inputs["edge_features"])
    nfi = _f(inputs["node_features_input"])
    coords = nfi[:, 0:3]
    cd_all = coords[start] - coords[end]
    cdn_all = np.sqrt((cd_all ** 2).sum(1)).astype(np.float32)
    deg = np.bincount(start, minlength=N).astype(np.float32)
    invcnt_all = (1.0 / np.maximum(deg, 1.0)).astype(np.float32)

    # uniform chunks-per-block across all cores/blocks (SPMD program shape)
    core = start // NPC
    lblk = (start - core * NPC) >> 7
    bc = np.bincount(core * NBLK + lblk, minlength=NCORES * NBLK)
    CPB = int(np.ceil(bc.max() / 128.0))
    NCHR = NBLK * CPB
    NCH = (NCHR + 7) // 8 * 8
    S = NCH // 8

    w = _prep_weights(inputs)
    in_maps = []
    for c in range(NCORES):
        d = _prep_core(c, start, end, ef, nfi, cd_all, cdn_all, invcnt_all, CPB, S)
        d.update(w)
        in_maps.append(d)

    key = (CPB, S)
    if _cache.get("key") != key:
        _cache["nc"] = _build_program(CPB, S)
        _cache["key"] = key
    nc = _cache["nc"]
    _cache["in_maps"] = in_maps
    res = run_bass_kernel_spmd(nc, in_maps, list(range(NCORES)))
    out = np.empty((N, 70), np.float32)
    for c in range(NCORES):
        out[c * NPC:(c + 1) * NPC] = res.results[c]["out"][0:NPC]
    return out


# revision 18
# speedup vs baseline: 1.2717x; 1.2717x over previous
"""EquivariantGraphConvolution (EGNN layer) on 8 Trainium2 NeuronCores.

Strategy (v2 — streamed, gather-free)
-------------------------------------
Nodes are range-partitioned across the 8 cores (6250 each); every edge is owned
by the core that owns its *start* node, so per-start segment sums are
core-local and no collective is needed.

Per core, edges are sorted by 128-node start block and padded per block to a
uniform CPB chunks of 128 edges.  The host pre-gathers both endpoints' node
features per edge and stages them as sequentially-streamed feature-major bf16
tensors (plus edge features / dist / coords-diff / lane ids), so the device
does ZERO indirect DMA — the edge MLP is pure dense matmul work:

  x1[128,512] = W1s_bd.T@nfs + W1e_bd.T@nfe + Wef.T@efcdn   (2 edge groups
  feature-stacked on partitions), SiLU chains for message/coords nets, a merged
  transpose+gate+coordw matmul per 128-edge chunk (K=128: msg|coord stacked),
  and a one-hot matmul segment-sum per chunk with the one-hot stationary
  (out is node-major [128,67] = 64 msg-agg + 3 coord-agg).

Node updates (velocity/node MLPs, coordinate update) run on-chip afterwards.
"""
import sys
sys.path.insert(0, "/opt/trn_rl_repo")
import contextlib
import numpy as np
import ml_dtypes

import concourse.bass as bass
import concourse.bacc as bacc
import concourse.mybir as mybir
import concourse.tile as tile
from concourse.bass_utils import run_bass_kernel_spmd

f32 = mybir.dt.float32
bf16 = mybir.dt.bfloat16
i32 = mybir.dt.int32
AF = mybir.ActivationFunctionType
OP = mybir.AluOpType
BF = ml_dtypes.bfloat16

# ---- problem constants (hardcoded per contract) ----
N = 50000
E = 1_000_000
H = 64
EF = 16
NCORES = 8
NPC = N // NCORES          # 6250 nodes per core
NBLK = 49                  # 128-node blocks per core (49*128 = 6272 >= 6250)
NPAD = NBLK * 128          # 6272

_cache = {}


def _f(x):
    return np.ascontiguousarray(x, np.float32)


def _bd(W):
    """[64,64] -> [128,128] block diagonal."""
    out = np.zeros((128, 128), np.float32)
    out[0:64, 0:64] = W
    out[64:128, 64:128] = W
    return out


def _prep_weights(inp):
    """Small weight/constant tensors, identical on all cores."""
    W_e1 = _f(inp["W_e1"])           # [145, 64]
    w = {}
    w["w1s"] = _bd(W_e1[0:64]).astype(BF)
    w["w1e"] = _bd(W_e1[64:128]).astype(BF)
    wef = np.zeros((34, 128), np.float32)
    wef[0:16, 0:64] = W_e1[129:145]
    wef[16:17, 0:64] = W_e1[128:129]
    wef[17:33, 64:128] = W_e1[129:145]
    wef[33:34, 64:128] = W_e1[128:129]
    w["wefcdn"] = wef.astype(BF)
    w["wde2"] = _bd(_f(inp["W_e2"])).astype(BF)
    w["wdc1"] = _bd(_f(inp["W_c1"])).astype(BF)
    W_i = _f(inp["W_i"]); W_c2 = _f(inp["W_c2"])
    ra = np.zeros((128, 65), np.float32)
    ra[0:64, 0:64] = np.eye(64); ra[0:64, 64:65] = W_i
    ra[64:128, 0:64] = np.eye(64); ra[64:128, 64:65] = W_i
    w["raw2"] = ra.astype(BF)
    wc2r = np.zeros((128, 1), np.float32)
    wc2r[0:64] = W_c2; wc2r[64:128] = W_c2
    w["wc2r"] = wc2r.astype(BF)
    w["wn1"] = _f(inp["W_n1"])       # [128, 64]
    w["wn2"] = _f(inp["W_n2"])       # [64, 64]
    w["wv1"] = _f(inp["W_v1"])       # [64, 64]
    w["wv2"] = _f(inp["W_v2"])       # [64, 1]
    w["eye64"] = np.eye(64, dtype=np.float32)
    w["eye128"] = np.eye(128, dtype=np.float32)
    b_e1 = _f(inp["b_e1"]); b_e2 = _f(inp["b_e2"]); b_c1 = _f(inp["b_c1"])
    w["be1s"] = np.concatenate([b_e1, b_e1]).reshape(128, 1)
    w["be2s"] = np.concatenate([b_e2, b_e2]).reshape(128, 1)
    w["bc1s"] = np.concatenate([b_c1, b_c1]).reshape(128, 1)
    w["bih"] = np.full((128, 1), 0.5 * float(np.asarray(inp["b_i"]).ravel()[0]), np.float32)
    w["bn1c"] = _f(inp["b_n1"]).reshape(64, 1)
    w["bn2c"] = _f(inp["b_n2"]).reshape(64, 1)
    w["bv1c"] = _f(inp["b_v1"]).reshape(64, 1)
    w["bv2c"] = np.full((1, 1), float(np.asarray(inp["b_v2"]).ravel()[0]), np.float32)
    return w


WSHAPES = {"w1s": [128, 128], "w1e": [128, 128], "wefcdn": [34, 128],
           "wde2": [128, 128], "wdc1": [128, 128], "raw2": [128, 65],
           "wc2r": [128, 1],
           "wn1": [128, 64], "wn2": [64, 64], "wv1": [64, 64],
           "wv2": [64, 1], "eye64": [64, 64], "eye128": [128, 128],
           "be1s": [128, 1], "be2s": [128, 1], "bc1s": [128, 1],
           "bih": [128, 1], "bn1c": [64, 1], "bn2c": [64, 1],
           "bv1c": [64, 1], "bv2c": [1, 1]}
BF_W = ("w1s", "w1e", "wefcdn", "wde2", "wdc1", "raw2", "wc2r")


def _prep_core(c, start, end, ef, nfi, cd_all, cdn_all, invcnt_all, CPB, S):
    """Per-core staged edge streams (sorted by start block, block-padded)."""
    NCH = S * 8
    NSLOT = NCH * 128
    lo, hi = c * NPC, (c + 1) * NPC
    sel = (start >= lo) & (start < hi)
    eo = np.nonzero(sel)[0]
    s_loc = (start[eo] - lo).astype(np.int64)
    blk = s_loc >> 7
    order = np.argsort(blk, kind="stable")
    eo = eo[order]; s_loc = s_loc[order]; blk = blk[order]
    counts = np.bincount(blk, minlength=NBLK)
    if counts.max() > CPB * 128:
        raise RuntimeError(f"block overflow: {counts.max()} > {CPB * 128}")
    starts = np.zeros(NBLK, np.int64)
    starts[1:] = np.cumsum(counts)[:-1]
    within = np.arange(len(eo)) - starts[blk]
    slots = blk * (CPB * 128) + within

    nf64 = nfi[:, 6:70]
    nfs_sl = np.zeros((NSLOT, 64), np.float32)
    nfe_sl = np.zeros((NSLOT, 64), np.float32)
    ef_sl = np.zeros((NSLOT, EF), np.float32)
    cdn_sl = np.zeros(NSLOT, np.float32)
    cd_sl = np.zeros((NSLOT, 3), np.float32)
    lid_sl = np.full(NSLOT, -1.0, np.float32)
    nfs_sl[slots] = nf64[start[eo]]
    nfe_sl[slots] = nf64[end[eo]]
    ef_sl[slots] = ef[eo]
    cdn_sl[slots] = cdn_all[eo]
    cd_sl[slots] = cd_all[eo]
    lid_sl[slots] = (s_loc & 127).astype(np.float32)

    d = {}
    # feature-major, 2 edge groups of 512 stacked on partitions;
    # nfse packs [nfs | nfe] along the free dim -> one DMA per supertile
    nfse = np.empty((S, 128, 1024), BF)
    v = nfs_sl.reshape(S, 2, 512, 64).transpose(0, 1, 3, 2)
    nfse[:, :, 0:512] = v.reshape(S, 128, 512)
    v = nfe_sl.reshape(S, 2, 512, 64).transpose(0, 1, 3, 2)
    nfse[:, :, 512:1024] = v.reshape(S, 128, 512)
    d["nfse"] = nfse
    eft = ef_sl.reshape(S, 2, 512, EF).transpose(0, 1, 3, 2)   # [S,2,16,512]
    cdnr = cdn_sl.reshape(S, 2, 512)
    d["efcdn"] = np.concatenate(
        [eft[:, 0], cdnr[:, 0][:, None, :], eft[:, 1], cdnr[:, 1][:, None, :]],
        axis=1).astype(BF)                                      # [S,34,512]
    cdlid = np.empty((S, 128, 8, 4), BF)
    cdlid[:, :, :, 0:3] = cd_sl.reshape(S, 8, 128, 3).transpose(0, 2, 1, 3)
    cdlid[:, :, :, 3] = lid_sl.reshape(S, 8, 128).transpose(0, 2, 1)
    d["cdlid"] = cdlid

    nm = np.zeros((NPAD, 70), np.float32)
    nm[0:NPC] = nfi[lo:hi]
    d["node_nm"] = nm.reshape(NBLK, 128, 70).transpose(1, 0, 2).reshape(128, NBLK * 70).copy()
    ic = np.ones(NPAD, np.float32)
    ic[0:NPC] = invcnt_all[lo:hi]
    d["invcnt"] = ic.reshape(NBLK, 128).T.copy()                # [128, NBLK]
    nl = np.zeros((64, NPAD), np.float32)
    nl[:, 0:NPC] = nfi[lo:hi, 6:70].T
    d["nfT_local"] = nl
    return d


def _build_program(CPB, S):
    NCH = S * 8
    nc = bacc.Bacc("TRN2", target_bir_lowering=False, debug=False,
                   enable_asserts=False, num_devices=NCORES)

    def din(name, shape, dt=f32):
        return nc.dram_tensor(name, list(shape), dt, kind="ExternalInput").ap()

    nfse_d = din("nfse", [S, 128, 1024], bf16)
    efcdn_d = din("efcdn", [S, 34, 512], bf16)
    cdlid_d = din("cdlid", [S, 128, 8, 4], bf16)
    invcnt_d = din("invcnt", [128, NBLK])
    node_nm_d = din("node_nm", [128, NBLK * 70])
    nfT_loc_d = din("nfT_local", [64, NPAD])
    wd = {n: din(n, WSHAPES[n], bf16 if n in BF_W else f32) for n in WSHAPES}
    out_d = nc.dram_tensor("out", [NPAD, 70], f32, kind="ExternalOutput").ap()

    with tile.TileContext(nc) as tc, contextlib.ExitStack() as ctx:
        wpool = ctx.enter_context(tc.tile_pool(name="w", bufs=1))
        wt = {}
        for n in WSHAPES:
            t = wpool.tile(WSHAPES[n], bf16 if n in BF_W else f32, name=f"wt_{n}")
            nc.sync.dma_start(t[:], wd[n][:])
            wt[n] = t
        iota32 = wpool.tile([128, 128], i32, name="iota32")
        nc.gpsimd.iota(iota32[:], pattern=[[1, 128]], base=0, channel_multiplier=0)
        iota = wpool.tile([128, 128], bf16, name="iota")
        nc.vector.tensor_copy(iota[:], iota32[:])
        node_nm = wpool.tile([128, NBLK * 70], f32, name="node_nm")
        nc.sync.dma_start(node_nm[:], node_nm_d[:])
        invcnt = wpool.tile([128, NBLK], f32, name="invcnt")
        nc.sync.dma_start(invcnt[:], invcnt_d[:])
        nfT_loc = wpool.tile([64, NPAD], f32, name="nfT_loc")
        nc.sync.dma_start(nfT_loc[:], nfT_loc_d[:])
        vscale = wpool.tile([128, NBLK], f32, name="vscale")
        aggsb = wpool.tile([128, NBLK * 67], f32, name="aggsb")  # node-major [n, blk*67]

        # ---------- Phase B: velocity MLP -> vscale [128, NBLK] ----------
        with tc.tile_pool(name="pb", bufs=2) as pb, \
             tc.tile_pool(name="pbp", bufs=2, space="PSUM") as pbp:
            tiles = [(j * 512, 512) for j in range(NPAD // 512)]
            if NPAD % 512:
                tiles.append((NPAD // 512 * 512, NPAD % 512))
            for (o, L) in tiles:
                vps = pbp.tile([64, L], f32, name=f"vps{o}", tag="vps")
                nc.tensor.matmul(vps[:], wt["wv1"][:], nfT_loc[:, o:o + L])
                vh = pb.tile([64, L], f32, name=f"vh{o}", tag="vh")
                nc.scalar.activation(vh[:], vps[:], AF.Silu, bias=wt["bv1c"][:])
                sps = pbp.tile([1, L], f32, name=f"sps{o}", tag="sps")
                nc.tensor.matmul(sps[:], wt["wv2"][:], vh[:])
                vsc = pb.tile([1, L], f32, name=f"vsc{o}", tag="vsc")
                nc.scalar.activation(vsc[:], sps[:], AF.Identity, bias=wt["bv2c"][:])
                for k in range(L // 128):
                    tp = pbp.tile([128, 1], f32, name=f"tp{o}_{k}", tag="tp")
                    nc.tensor.transpose(tp[:], vsc[:, k * 128:(k + 1) * 128],
                                        wt["eye64"][0:1, 0:1])
                    nc.vector.tensor_copy(vscale[:, o // 128 + k:o // 128 + k + 1], tp[:])

        # ---------- Edge sweep ----------
        with tc.tile_pool(name="pin", bufs=6) as pin, \
             tc.tile_pool(name="pmid", bufs=4) as pmid, \
             tc.tile_pool(name="px", bufs=2, space="PSUM") as px, \
             tc.tile_pool(name="pm", bufs=2, space="PSUM") as pm, \
             tc.tile_pool(name="pst", bufs=2, space="PSUM") as pst, \
             tc.tile_pool(name="pagg", bufs=2, space="PSUM") as pagg:
            aggN = None
            live = {}
            for s in range(S + 1):
              if s < S:
                nfse = pin.tile([128, 1024], bf16, name=f"nfse{s}", tag="nfse")
                nc.sync.dma_start(nfse[:], nfse_d[s])
                eft = pin.tile([34, 512], bf16, name=f"eft{s}", tag="eft")
                nc.sync.dma_start(eft[:], efcdn_d[s])
                cdlid = pin.tile([128, 8, 4], bf16, name=f"cdlid{s}", tag="cdlid")
                nc.sync.dma_start(cdlid[:], cdlid_d[s])

                oht = pmid.tile([128, 8, 128], bf16, name=f"oht{s}", tag="oht")
                nc.vector.tensor_tensor(
                    oht[:], iota[:].unsqueeze(1).broadcast_to([128, 8, 128]),
                    cdlid[:, :, 3:4].broadcast_to([128, 8, 128]), OP.is_equal)

                x1 = px.tile([128, 512], f32, name=f"x1{s}", tag="x1")
                nc.tensor.matmul(x1[:], wt["w1s"][:], nfse[:, 0:512],
                                 start=True, stop=False)
                nc.tensor.matmul(x1[:], wt["w1e"][:], nfse[:, 512:1024],
                                 start=False, stop=False)
                nc.tensor.matmul(x1[:], wt["wefcdn"][:], eft[:], start=False, stop=True)
                h1 = pmid.tile([128, 512], bf16, name=f"h1{s}", tag="h1")
                nc.scalar.activation(h1[:], x1[:], AF.Silu, bias=wt["be1s"][:])
                mp = pm.tile([128, 512], f32, name=f"mp{s}", tag="mm2")
                nc.tensor.matmul(mp[:], wt["wde2"][:], h1[:])
                msgT = pmid.tile([128, 512], bf16, name=f"msgT{s}", tag="msgT")
                nc.scalar.activation(msgT[:], mp[:], AF.Silu, bias=wt["be2s"][:])
                cp = pm.tile([128, 512], f32, name=f"cp{s}", tag="mm2")
                nc.tensor.matmul(cp[:], wt["wdc1"][:], msgT[:])
                chT = pmid.tile([128, 512], bf16, name=f"chT{s}", tag="chT")
                nc.scalar.activation(chT[:], cp[:], AF.Silu, bias=wt["bc1s"][:])

                rgc = pmid.tile([128, 8, 67], bf16, name=f"rgc{s}", tag="rgc")
                for g in range(2):
                    rows = slice(g * 64, g * 64 + 64)
                    st = pst.tile([128, 4, 66], f32, name=f"st{s}_{g}", tag="st")
                    for c4 in range(4):
                        cc = slice(c4 * 128, (c4 + 1) * 128)
                        nc.tensor.matmul(st[:, c4, 0:65], msgT[rows, cc],
                                         wt["raw2"][rows, :], start=True, stop=True)
                        nc.tensor.matmul(st[:, c4, 65:66], chT[rows, cc],
                                         wt["wc2r"][rows, :], start=True, stop=True)
                    tnh = pmid.tile([128, 4], f32, name=f"tnh{s}_{g}", tag="tnh")
                    nc.scalar.activation(tnh[:], st[:, :, 64:65].squeeze(2),
                                         AF.Tanh, bias=wt["bih"][:], scale=0.5)
                    gate = pmid.tile([128, 4], f32, name=f"gt{s}_{g}", tag="gate")
                    nc.vector.tensor_scalar(out=gate[:], in0=tnh[:], scalar1=1.0,
                                            scalar2=0.5, op0=OP.add, op1=OP.mult)
                    gsl = slice(g * 4, g * 4 + 4)
                    nc.vector.tensor_tensor(
                        rgc[:, gsl, 0:64], st[:, :, 0:64],
                        gate[:].unsqueeze(2).broadcast_to([128, 4, 64]), OP.mult)
                    nc.vector.tensor_tensor(
                        rgc[:, gsl, 64:67], cdlid[:, gsl, 0:3],
                        st[:, :, 65:66].broadcast_to([128, 4, 3]), OP.mult)
                live[s] = (oht, rgc)

              # scatter for supertile s-1: rgc/oht are a full stage old, so
              # the PE never stalls on the DVE gate/copy chain
              if s >= 1:
                oht1, rgc1 = live.pop(s - 1)
                for k in range(8):
                    gc = (s - 1) * 8 + k
                    vb = gc // CPB
                    if vb >= NBLK:
                        continue
                    pos = gc % CPB
                    if pos == 0:
                        aggN = pagg.tile([128, 128], f32, name=f"agg{vb}", tag="agg")
                    nc.tensor.matmul(aggN[:, 0:67], oht1[:, k, :], rgc1[:, k, :],
                                     start=(pos == 0), stop=(pos == CPB - 1),
                                     skip_group_check=True)
                    if pos == CPB - 1:
                        nc.vector.tensor_copy(aggsb[:, vb * 67:(vb + 1) * 67],
                                              aggN[:, 0:67])

        # ---------- Phase C: node update + output ----------
        with tc.tile_pool(name="pc", bufs=3) as pc, \
             tc.tile_pool(name="pcp", bufs=2, space="PSUM") as pcp:
            b0 = 0
            while b0 < NBLK:
                BB = min(4, NBLK - b0)
                L = BB * 128
                xnT = pc.tile([128, BB, 128], f32, name=f"xnT{b0}", tag="xnT")
                nc.vector.tensor_copy(
                    xnT[0:64, :, :],
                    nfT_loc[:, b0 * 128:b0 * 128 + L].rearrange(
                        "p (b n) -> p b n", b=BB))
                atp = pcp.tile([64, BB, 128], f32, name=f"atp{b0}", tag="atp")
                for j in range(BB):
                    nc.tensor.transpose(
                        atp[:, j, :],
                        aggsb[:, (b0 + j) * 67:(b0 + j) * 67 + 64],
                        wt["eye128"][:])
                nc.vector.tensor_copy(xnT[64:128, :, :], atp[:])
                n1 = pcp.tile([64, BB, 128], f32, name=f"n1{b0}", tag="n1")
                nc.tensor.matmul(n1[:].rearrange("p b n -> p (b n)"), wt["wn1"][:],
                                 xnT[:].rearrange("p b n -> p (b n)"))
                hn = pc.tile([64, BB, 128], f32, name=f"hn{b0}", tag="hn")
                nc.scalar.activation(hn[:].rearrange("p b n -> p (b n)"),
                                     n1[:].rearrange("p b n -> p (b n)"),
                                     AF.Silu, bias=wt["bn1c"][:])
                n2 = pcp.tile([64, BB, 128], f32, name=f"n2{b0}", tag="n2")
                nc.tensor.matmul(n2[:].rearrange("p b n -> p (b n)"), wt["wn2"][:],
                                 hn[:].rearrange("p b n -> p (b n)"))
                hn2 = pc.tile([64, BB, 128], f32, name=f"hn2{b0}", tag="hn2")
                nc.scalar.activation(hn2[:].rearrange("p b n -> p (b n)"),
                                     n2[:].rearrange("p b n -> p (b n)"),
                                     AF.Identity, bias=wt["bn2c"][:])
                ndel = pcp.tile([128, BB, 64], f32, name=f"ndel{b0}", tag="ndel")
                for j in range(BB):
                    nc.tensor.transpose(ndel[:, j, :], hn2[:, j, :], wt["eye64"][:])
                nmb = node_nm[:, b0 * 70:(b0 + BB) * 70].rearrange(
                    "p (b f) -> p b f", b=BB)
                t1 = pc.tile([128, BB, 3], f32, name=f"t1{b0}", tag="t1")
                nc.vector.tensor_tensor(
                    t1[:],
                    aggsb[:, b0 * 67:(b0 + BB) * 67].rearrange(
                        "p (b f) -> p b f", b=BB)[:, :, 64:67],
                    invcnt[:, b0:b0 + BB].unsqueeze(2).broadcast_to([128, BB, 3]),
                    OP.mult)
                t2 = pc.tile([128, BB, 3], f32, name=f"t2{b0}", tag="t2")
                nc.vector.tensor_tensor(
                    t2[:], nmb[:, :, 3:6],
                    vscale[:, b0:b0 + BB].unsqueeze(2).broadcast_to([128, BB, 3]),
                    OP.mult)
                t3 = pc.tile([128, BB, 3], f32, name=f"t3{b0}", tag="t3")
                nc.vector.tensor_tensor(t3[:], t1[:], t2[:], OP.add)
                ot = pc.tile([128, BB, 70], f32, name=f"ot{b0}", tag="ot")
                nc.vector.tensor_tensor(ot[:, :, 0:3], t3[:], nmb[:, :, 0:3], OP.add)
                nc.vector.tensor_copy(ot[:, :, 3:6], nmb[:, :, 3:6])
                nc.vector.tensor_tensor(ot[:, :, 6:70], nmb[:, :, 6:70], ndel[:],
                                        OP.add)
                nc.sync.dma_start(
                    out_d[b0 * 128:(b0 + BB) * 128, :].rearrange(
                        "(b p) f -> p b f", p=128),
                    ot[:])
                b0 += BB

    nc.compile()
    return nc


def kernel(**inputs):
    ei = np.asarray(inputs["edge_indices"])
    start = ei[0].astype(np.int64)
    end = ei[1].astype(np.int64)
    ef = _f(inputs["edge_features"])
    nfi = _f(inputs["node_features_input"])
    coords = nfi[:, 0:3]
    cd_all = coords[start] - coords[end]
    cdn_all = np.sqrt((cd_all ** 2).sum(1)).astype(np.float32)
    deg = np.bincount(start, minlength=N).astype(np.float32)
    invcnt_all = (1.0 / np.maximum(deg, 1.0)).astype(np.float32)

    # uniform chunks-per-block across all cores/blocks (SPMD program shape)
    core = start // NPC
    lblk = (start - core * NPC) >> 7
    bc = np.bincount(core * NBLK + lblk, minlength=NCORES * NBLK)
    CPB = int(np.ceil(bc.max() / 128.0))
    NCHR = NBLK * CPB
    NCH = (NCHR + 7) // 8 * 8
    S = NCH // 8

    w = _prep_weights(inputs)
    in_maps = []
    for c in range(NCORES):
        d = _prep_core(c, start, end, ef, nfi, cd_all, cdn_all, invcnt_all, CPB, S)
        d.update(w)
        in_maps.append(d)

    key = (CPB, S)
    if _cache.get("key") != key:
        _cache["nc"] = _build_program(CPB, S)
        _cache["key"] = key
    nc = _cache["nc"]
    _cache["in_maps"] = in_maps
    res = run_bass_kernel_spmd(nc, in_maps, list(range(NCORES)))
    out = np.empty((N, 70), np.float32)
    for c in range(NCORES):
        out[c * NPC:(c + 1) * NPC] = res.results[c]["out"][0:NPC]
    return out
